# revision 22
# baseline (speedup 1.0000x reference)
"""HQQ+SVD quantized linear for TRN2, tensor-parallel over out_features on 8 cores.

Math (approximates reference.py within ~8.5e-3 max-rel, gate is 2e-2):
  reference: W_f = (w-zp)*sc + up@dn;  out = (x_q @ Wq8.T)*sx*sw + bias
  kernel:    out = xqp @ W_f.T + bias  with xqp = fp16(x_q*sx)  (x-quant replicated
             exactly on host; the reference's W-requant noise is the only deviation)

Device decomposition per o-tile (128 rows):
  P_g[o,t]  = sum_{k in g} wT[k,o]*xqp[t,k]     (PE, raw int-valued fp16 weights)
  P_31     += (zp*sc@-sxg + bias + up@xd)/sc31  (PE; consecutive accum chain)
  Pc_g      = fp16(P_g)                         (ACT psum->sbuf copy)
  S_g       = Pc_g * scB_g                      (DVE fp16 TT, 2x packed)
  out[o,t]  = sum_g S_g                         (DVE fp16 pairwise tree)
"""
import sys
sys.path.insert(0, "/opt/trn_rl_repo")

import numpy as np
import concourse.bass as bass
import concourse.bacc as bacc
import concourse.tile as tile
import concourse.mybir as mybir

F32 = mybir.dt.float32
F16 = mybir.dt.float16
F8 = mybir.dt.float8e4
ALU = mybir.AluOpType
AX = mybir.AxisListType

OUT, N_GROUPS, GROUP = 11008, 32, 128
IN = N_GROUPS * GROUP            # 4096
RANK = 128
T = 64
NCORES = 8
SHARD = OUT // NCORES            # 1376
PAD = 1408                       # 11 * 128
NTILES = PAD // 128              # 11
HALF = N_GROUPS // 2             # 16 groups per psum half

_nc_cache = {}


def _build():
    if "nc" in _nc_cache:
        return _nc_cache["nc"]
    nc = bacc.Bacc("TRN2", target_bir_lowering=False, debug=False)

    w_d = nc.dram_tensor("w", [PAD, IN], F8, kind="ExternalInput")
    xqp_d = nc.dram_tensor("xqp", [128, N_GROUPS * T], F16, kind="ExternalInput")
    sc2_d = nc.dram_tensor("sc2", [128, NTILES * N_GROUPS], F16,
                           kind="ExternalInput")
    # zpscT row 32 carries bias/sc31 (paired with nsxgT row 32 = +1)
    zpscT_d = nc.dram_tensor("zpscT", [N_GROUPS + 1, PAD], F16, kind="ExternalInput")
    nsxgT_d = nc.dram_tensor("nsxgT", [N_GROUPS + 1, T], F16, kind="ExternalInput")
    upT_d = nc.dram_tensor("upT", [RANK, PAD], F16, kind="ExternalInput")
    dnT_d = nc.dram_tensor("dnT", [128, IN], F16, kind="ExternalInput")
    out_d = nc.dram_tensor("out", [PAD, T], F16, kind="ExternalOutput")

    with tile.TileContext(nc) as tc:
        with (
            tc.tile_pool(name="const", bufs=1) as cp,
            tc.tile_pool(name="work", bufs=2) as wp,
            tc.tile_pool(name="pp", bufs=4, space="PSUM") as pp,
        ):
            # ---- constants needed by tile-0's first matmuls go first
            xqp_t = cp.tile([128, N_GROUPS * T], F16, tag="xqp")
            nc.sync.dma_start(out=xqp_t[:], in_=xqp_d[:])
            dnT_t = cp.tile([128, IN], F16, tag="dnT")
            nc.sync.dma_start(out=dnT_t[:], in_=dnT_d[:])
            # tile 0's weights before the bulky remaining constants
            w_tiles = [wp.tile([128, IN], F8, tag="wt", name=f"wt{k}")
                       for k in range(2)]
            nc.sync.dma_start(out=w_tiles[0][:], in_=w_d[0:128, :])

            zpscT_t = cp.tile([N_GROUPS + 1, PAD], F16, tag="zpscT")
            nc.sync.dma_start(out=zpscT_t[:], in_=zpscT_d[:])
            nsxgT_t = cp.tile([N_GROUPS + 1, T], F16, tag="nsxgT")
            nc.sync.dma_start(out=nsxgT_t[:], in_=nsxgT_d[:])
            upT_t = cp.tile([RANK, PAD], F16, tag="upT")
            nc.sync.dma_start(out=upT_t[:], in_=upT_d[:])
            # scB expanded on-chip by the otherwise-idle gpsimd engine
            # (HBM only carries the [128,352] scale block)
            sc2_t = cp.tile([128, NTILES * N_GROUPS], F16, tag="sc2")
            nc.sync.dma_start(out=sc2_t[:], in_=sc2_d[:])
            scB_t = cp.tile([128, NTILES * N_GROUPS * T], F16, tag="scB")
            CH = N_GROUPS * T
            for ci in range(NTILES):
                src_ap = sc2_t[:, ci * N_GROUPS:(ci + 1) * N_GROUPS]
                src_ap = src_ap.unsqueeze(2).broadcast_to([128, N_GROUPS, T])
                dst_ap = scB_t[:, ci * CH:(ci + 1) * CH].rearrange(
                    "p (g t) -> p g t", t=T)
                nc.gpsimd.tensor_copy(dst_ap, src_ap)

            # ---- tile 0 half 0 matmuls first so PE isn't blocked on dnT
            P00 = pp.tile([128, HALF * T], F32, tag="p", name="P00")
            for j in range(HALF):
                nc.tensor.matmul(P00[:, j * T:(j + 1) * T],
                                 w_tiles[0][:, j * 128:(j + 1) * 128],
                                 xqp_t[:, j * T:(j + 1) * T],
                                 start=True, stop=True,
                                 skip_group_check=True)

            # ---- xd[r,t] = sum_g dn_g.T @ xqp_g  (rank-128 SVD intermediate)
            xd_ps = pp.tile([128, HALF * T], F32, tag="p")
            for g in range(N_GROUPS):
                nc.tensor.matmul(xd_ps[:, :T], dnT_t[:, g * 128:(g + 1) * 128],
                                 xqp_t[:, g * T:(g + 1) * T],
                                 start=(g == 0), stop=(g == N_GROUPS - 1))
            xd_t = cp.tile([128, T], F16, tag="xd")
            nc.vector.tensor_copy(xd_t[:], xd_ps[:, :T])

            # ---- per o-tile pipeline
            for i in range(NTILES):
                osl = slice(i * 128, (i + 1) * 128)
                w_t = w_tiles[i % 2] if i < 2 else wp.tile([128, IN], F8,
                                                           tag="wt")
                if i > 0:
                    nc.sync.dma_start(out=w_t[:], in_=w_d[osl, :])

                S_t = wp.tile([128, N_GROUPS * T], F16, tag="st")
                R_t = wp.tile([128, N_GROUPS * T // 2], F16, tag="rt")
                for h in range(2):
                    if (i, h) == (0, 0):
                        P = P00
                    else:
                        P = pp.tile([128, HALF * T], F32, tag="p")
                    for j in range(HALF):
                        if (i, h) == (0, 0):
                            break
                        g = h * HALF + j
                        qchain = (h, j) == (1, HALF - 1)
                        nc.tensor.matmul(P[:, j * T:(j + 1) * T],
                                         w_t[:, g * 128:(g + 1) * 128],
                                         xqp_t[:, g * T:(g + 1) * T],
                                         start=True, stop=not qchain,
                                         skip_group_check=True)
                        if qchain:
                            # zp*sc+bias (K=33) and SVD (K=128) accumulate
                            # into P[g=31] consecutively (host pre-divided
                            # them by sc[o,31])
                            jsl = slice(j * T, (j + 1) * T)
                            nc.tensor.matmul(P[:, jsl], zpscT_t[:, osl],
                                             nsxgT_t[:],
                                             start=False, stop=False,
                                             skip_group_check=True)
                            nc.tensor.matmul(P[:, jsl], upT_t[:, osl],
                                             xd_t[:],
                                             start=False, stop=True,
                                             skip_group_check=True)
                    # ACT: psum -> sbuf fp16; DVE: one contiguous fp16 2x TT
                    Pc = wp.tile([128, HALF * T], F16, tag="pc")
                    nc.scalar.copy(Pc[:], P[:])
                    hsl = slice(h * HALF * T, (h + 1) * HALF * T)
                    nc.vector.tensor_tensor(
                        S_t[:, hsl], Pc[:],
                        scB_t[:, i * N_GROUPS * T + h * HALF * T:
                              i * N_GROUPS * T + (h + 1) * HALF * T],
                        ALU.mult)

                # out[o,t] = sum_g S[o,(g t)]: contiguous fp16 pairwise tree
                nc.vector.tensor_tensor(R_t[:, :1024], S_t[:, :1024],
                                        S_t[:, 1024:], ALU.add)
                nc.vector.tensor_tensor(S_t[:, :512], R_t[:, :512],
                                        R_t[:, 512:1024], ALU.add)
                nc.vector.tensor_tensor(R_t[:, :256], S_t[:, :256],
                                        S_t[:, 256:512], ALU.add)
                nc.vector.tensor_tensor(S_t[:, :128], R_t[:, :128],
                                        R_t[:, 128:256], ALU.add)
                out_t = wp.tile([128, T], F16, tag="out")
                nc.vector.tensor_tensor(out_t[:], S_t[:, :T],
                                        S_t[:, T:2 * T], ALU.add)
                nc.gpsimd.dma_start(out=out_d[osl, :], in_=out_t[:])

    nc.compile()
    _nc_cache["nc"] = nc
    return nc


def _prep_inputs(x, weight, scale, zero_point, svd_up, svd_down, bias):
    x = np.asarray(x, dtype=np.float32)
    weight = np.asarray(weight)
    scale = np.asarray(scale, dtype=np.float32)
    zero_point = np.asarray(zero_point, dtype=np.float32)
    svd_up = np.asarray(svd_up, dtype=np.float32)
    svd_down = np.asarray(svd_down, dtype=np.float32)
    bias = np.asarray(bias, dtype=np.float32)

    # exact replication of reference's x-quant, then fold sx back in (fp16)
    xt = x.reshape(-1, IN)
    sx = (np.max(np.abs(xt), axis=1, keepdims=True) / np.float32(127.0))
    xq = np.clip(np.round(xt / sx), -128, 127).astype(np.float32)
    xqp = (xq * sx).astype(np.float16)                     # [T, IN]
    # xqp_d[p, g*T+t] = xqp[t, g*128+p]
    xqp_l = np.ascontiguousarray(
        xqp.T.reshape(N_GROUPS, 128, T).transpose(1, 0, 2).reshape(128, N_GROUPS * T))
    # -sxg[t,g] = -sum_{k in g} xqp[t,k], exact fp32 sum of the fp16 values;
    # row 32 pairs with the bias row of zpscT
    sxg = xqp.astype(np.float32).reshape(T, N_GROUPS, 128).sum(axis=2)   # [T,32]
    nsxgT = np.concatenate([-sxg.T, np.ones((1, T), np.float32)],
                           axis=0).astype(np.float16)                    # [33,T]
    # dnT_d[p, g*128+r] = dn[r, g*128+p]
    dnT = np.ascontiguousarray(
        svd_down.T.reshape(N_GROUPS, 128, RANK).transpose(1, 0, 2).reshape(128, IN)
    ).astype(np.float16)

    npad = PAD - SHARD
    in_maps = []
    for c in range(NCORES):
        sl = slice(c * SHARD, (c + 1) * SHARD)
        import ml_dtypes
        f8 = ml_dtypes.float8_e4m3
        w_c = np.concatenate([weight[sl].astype(f8),
                              np.zeros((npad, N_GROUPS, GROUP), f8)], axis=0)
        # w_d[i*128+p, g*128+c2] = w[i*128+c2, g, p]
        w_l = np.ascontiguousarray(
            w_c.reshape(NTILES, 128, N_GROUPS, 128).transpose(0, 3, 2, 1)
            .reshape(PAD, IN))
        sc_c = np.concatenate([scale[sl], np.zeros((npad, N_GROUPS), np.float32)], 0)
        # sc_l[p, i*32+g] = sc[i*128+p, g]; scB repeats each column T times
        sc_l = np.ascontiguousarray(
            sc_c.reshape(NTILES, 128, N_GROUPS).transpose(1, 0, 2)
            .reshape(128, NTILES * N_GROUPS))
        sc2 = sc_l.astype(np.float16)                          # [128, 352]
        # zp/svd/bias terms ride in P[g=31], pre-divided by sc[o,31]
        sc0 = sc_c[:, N_GROUPS - 1].copy()
        sc0[sc0 == 0] = 1.0
        zp_c = np.concatenate([zero_point[sl],
                               np.zeros((npad, N_GROUPS), np.float32)], 0)
        bias_c = np.concatenate([bias[sl], np.zeros(npad, np.float32)])
        zpscT = np.ascontiguousarray(
            np.concatenate([(zp_c * sc_c) / sc0[:, None],
                            (bias_c / sc0)[:, None]], axis=1).T
        ).astype(np.float16)                                              # [33,PAD]
        up_c = np.concatenate([svd_up[sl], np.zeros((npad, RANK), np.float32)], 0)
        upT = np.ascontiguousarray((up_c / sc0[:, None]).T).astype(np.float16)
        in_maps.append(dict(
            w=w_l, xqp=xqp_l, sc2=sc2, zpscT=zpscT, nsxgT=nsxgT,
            upT=upT, dnT=dnT))
    return in_maps


def kernel(x, weight, scale, zero_point, svd_up, svd_down, bias):
    nc = _build()
    in_maps = _prep_inputs(x, weight, scale, zero_point, svd_up, svd_down, bias)
    _nc_cache["last_in_maps"] = in_maps
    from concourse.bass_utils import run_bass_kernel_spmd
    res = run_bass_kernel_spmd(nc, in_maps, core_ids=list(range(NCORES)))
    outs = [r["out"][:SHARD].astype(np.float32) for r in res.results]
    full = np.concatenate(outs, axis=0)                         # [OUT, T]
    return np.ascontiguousarray(full.T)[None].astype(np.float32)  # [1, T, OUT]


# revision 23
# speedup vs baseline: 1.7886x; 1.7886x over previous
"""HQQ+SVD quantized linear for TRN2, tensor-parallel over out_features on 8 cores.

Math (approximates reference.py within ~8.5e-3 max-rel, gate is 2e-2):
  reference: W_f = (w-zp)*sc + up@dn;  out = (x_q @ Wq8.T)*sx*sw + bias
  kernel:    out = xqp @ W_f.T + bias  with xqp = fp16(x_q*sx)  (x-quant replicated
             exactly on host; the reference's W-requant noise is the only deviation)

Device decomposition per o-tile (128 rows):
  P_g[o,t]  = sum_{k in g} wT[k,o]*xqp[t,k]     (PE, raw int-valued fp16 weights)
  P_31     += (zp*sc@-sxg + bias + up@xd)/sc31  (PE; consecutive accum chain)
  Pc_g      = fp16(P_g)                         (ACT psum->sbuf copy)
  S_g       = Pc_g * scB_g                      (DVE fp16 TT, 2x packed)
  out[o,t]  = sum_g S_g                         (DVE fp16 pairwise tree)
"""
import sys
sys.path.insert(0, "/opt/trn_rl_repo")

import numpy as np
import concourse.bass as bass
import concourse.bacc as bacc
import concourse.tile as tile
import concourse.mybir as mybir

F32 = mybir.dt.float32
F16 = mybir.dt.float16
F8 = mybir.dt.float8e4
ALU = mybir.AluOpType
AX = mybir.AxisListType

OUT, N_GROUPS, GROUP = 11008, 32, 128
IN = N_GROUPS * GROUP            # 4096
RANK = 128
T = 64
NCORES = 8
SHARD = OUT // NCORES            # 1376
PAD = 1408                       # 11 * 128
NTILES = PAD // 128              # 11
HALF = N_GROUPS // 2             # 16 groups per psum half

_nc_cache = {}


def _build():
    if "nc" in _nc_cache:
        return _nc_cache["nc"]
    nc = bacc.Bacc("TRN2", target_bir_lowering=False, debug=False)

    w_d = nc.dram_tensor("w", [PAD, IN], F8, kind="ExternalInput")
    xqp_d = nc.dram_tensor("xqp", [128, N_GROUPS * T], F16, kind="ExternalInput")
    scB_d = nc.dram_tensor("scB", [128, NTILES * N_GROUPS * T], F16,
                           kind="ExternalInput")
    # zpscT row 32 carries bias/sc31 (paired with nsxgT row 32 = +1)
    zpscT_d = nc.dram_tensor("zpscT", [N_GROUPS + 1, PAD], F16, kind="ExternalInput")
    nsxgT_d = nc.dram_tensor("nsxgT", [N_GROUPS + 1, T], F16, kind="ExternalInput")
    upT_d = nc.dram_tensor("upT", [RANK, PAD], F16, kind="ExternalInput")
    dnT_d = nc.dram_tensor("dnT", [128, IN], F16, kind="ExternalInput")
    out_d = nc.dram_tensor("out", [PAD, T], F16, kind="ExternalOutput")

    with tile.TileContext(nc) as tc:
        with (
            tc.tile_pool(name="const", bufs=1) as cp,
            tc.tile_pool(name="work", bufs=2) as wp,
            tc.tile_pool(name="pp", bufs=4, space="PSUM") as pp,
        ):
            # ---- constants needed by tile-0's first matmuls go first
            xqp_t = cp.tile([128, N_GROUPS * T], F16, tag="xqp")
            nc.sync.dma_start(out=xqp_t[:], in_=xqp_d[:])
            dnT_t = cp.tile([128, IN], F16, tag="dnT")
            nc.sync.dma_start(out=dnT_t[:], in_=dnT_d[:])
            # tile 0's weights before the bulky remaining constants
            w_tiles = [wp.tile([128, IN], F8, tag="wt", name=f"wt{k}")
                       for k in range(2)]
            nc.sync.dma_start(out=w_tiles[0][:], in_=w_d[0:128, :])

            zpscT_t = cp.tile([N_GROUPS + 1, PAD], F16, tag="zpscT")
            nc.sync.dma_start(out=zpscT_t[:], in_=zpscT_d[:])
            nsxgT_t = cp.tile([N_GROUPS + 1, T], F16, tag="nsxgT")
            nc.sync.dma_start(out=nsxgT_t[:], in_=nsxgT_d[:])
            upT_t = cp.tile([RANK, PAD], F16, tag="upT")
            nc.sync.dma_start(out=upT_t[:], in_=upT_d[:])
            # scB host-expanded, fetched in per-tile chunks so tile 0
            # isn't gated on the full 5.8MB
            scB_t = cp.tile([128, NTILES * N_GROUPS * T], F16, tag="scB")
            CH = N_GROUPS * T
            for ci in range(NTILES):
                nc.sync.dma_start(out=scB_t[:, ci * CH:(ci + 1) * CH],
                                  in_=scB_d[:, ci * CH:(ci + 1) * CH])

            # ---- tile 0 half 0 matmuls first so PE isn't blocked on dnT
            P00 = pp.tile([128, HALF * T], F32, tag="p", name="P00")
            for j in range(HALF):
                nc.tensor.matmul(P00[:, j * T:(j + 1) * T],
                                 w_tiles[0][:, j * 128:(j + 1) * 128],
                                 xqp_t[:, j * T:(j + 1) * T],
                                 start=True, stop=True,
                                 skip_group_check=True)

            # ---- xd[r,t] = sum_g dn_g.T @ xqp_g  (rank-128 SVD intermediate)
            xd_ps = pp.tile([128, HALF * T], F32, tag="p")
            for g in range(N_GROUPS):
                nc.tensor.matmul(xd_ps[:, :T], dnT_t[:, g * 128:(g + 1) * 128],
                                 xqp_t[:, g * T:(g + 1) * T],
                                 start=(g == 0), stop=(g == N_GROUPS - 1))
            xd_t = cp.tile([128, T], F16, tag="xd")
            nc.vector.tensor_copy(xd_t[:], xd_ps[:, :T])

            # ---- per o-tile pipeline
            for i in range(NTILES):
                osl = slice(i * 128, (i + 1) * 128)
                w_t = w_tiles[i % 2] if i < 2 else wp.tile([128, IN], F8,
                                                           tag="wt")
                if i > 0:
                    nc.sync.dma_start(out=w_t[:], in_=w_d[osl, :])

                S_t = wp.tile([128, N_GROUPS * T], F16, tag="st")
                R_t = wp.tile([128, N_GROUPS * T // 2], F16, tag="rt")
                for h in range(2):
                    if (i, h) == (0, 0):
                        P = P00
                    else:
                        P = pp.tile([128, HALF * T], F32, tag="p")
                    for j in range(HALF):
                        if (i, h) == (0, 0):
                            break
                        g = h * HALF + j
                        qchain = (h, j) == (1, HALF - 1)
                        nc.tensor.matmul(P[:, j * T:(j + 1) * T],
                                         w_t[:, g * 128:(g + 1) * 128],
                                         xqp_t[:, g * T:(g + 1) * T],
                                         start=True, stop=not qchain,
                                         skip_group_check=True)
                        if qchain:
                            # zp*sc+bias (K=33) and SVD (K=128) accumulate
                            # into P[g=31] consecutively (host pre-divided
                            # them by sc[o,31])
                            jsl = slice(j * T, (j + 1) * T)
                            nc.tensor.matmul(P[:, jsl], zpscT_t[:, osl],
                                             nsxgT_t[:],
                                             start=False, stop=False,
                                             skip_group_check=True)
                            nc.tensor.matmul(P[:, jsl], upT_t[:, osl],
                                             xd_t[:],
                                             start=False, stop=True,
                                             skip_group_check=True)
                    # ACT: psum -> sbuf fp16; DVE: one contiguous fp16 2x TT
                    Pc = wp.tile([128, HALF * T], F16, tag="pc")
                    nc.scalar.copy(Pc[:], P[:])
                    hsl = slice(h * HALF * T, (h + 1) * HALF * T)
                    nc.vector.tensor_tensor(
                        S_t[:, hsl], Pc[:],
                        scB_t[:, i * N_GROUPS * T + h * HALF * T:
                              i * N_GROUPS * T + (h + 1) * HALF * T],
                        ALU.mult)

                # out[o,t] = sum_g S[o,(g t)]: contiguous fp16 pairwise tree
                nc.vector.tensor_tensor(R_t[:, :1024], S_t[:, :1024],
                                        S_t[:, 1024:], ALU.add)
                nc.vector.tensor_tensor(S_t[:, :512], R_t[:, :512],
                                        R_t[:, 512:1024], ALU.add)
                nc.vector.tensor_tensor(R_t[:, :256], S_t[:, :256],
                                        S_t[:, 256:512], ALU.add)
                nc.vector.tensor_tensor(S_t[:, :128], R_t[:, :128],
                                        R_t[:, 128:256], ALU.add)
                out_t = wp.tile([128, T], F16, tag="out")
                nc.vector.tensor_tensor(out_t[:], S_t[:, :T],
                                        S_t[:, T:2 * T], ALU.add)
                nc.gpsimd.dma_start(out=out_d[osl, :], in_=out_t[:])

    nc.compile()
    _nc_cache["nc"] = nc
    return nc


def _prep_inputs(x, weight, scale, zero_point, svd_up, svd_down, bias):
    x = np.asarray(x, dtype=np.float32)
    weight = np.asarray(weight)
    scale = np.asarray(scale, dtype=np.float32)
    zero_point = np.asarray(zero_point, dtype=np.float32)
    svd_up = np.asarray(svd_up, dtype=np.float32)
    svd_down = np.asarray(svd_down, dtype=np.float32)
    bias = np.asarray(bias, dtype=np.float32)

    # exact replication of reference's x-quant, then fold sx back in (fp16)
    xt = x.reshape(-1, IN)
    sx = (np.max(np.abs(xt), axis=1, keepdims=True) / np.float32(127.0))
    xq = np.clip(np.round(xt / sx), -128, 127).astype(np.float32)
    xqp = (xq * sx).astype(np.float16)                     # [T, IN]
    # xqp_d[p, g*T+t] = xqp[t, g*128+p]
    xqp_l = np.ascontiguousarray(
        xqp.T.reshape(N_GROUPS, 128, T).transpose(1, 0, 2).reshape(128, N_GROUPS * T))
    # -sxg[t,g] = -sum_{k in g} xqp[t,k], exact fp32 sum of the fp16 values;
    # row 32 pairs with the bias row of zpscT
    sxg = xqp.astype(np.float32).reshape(T, N_GROUPS, 128).sum(axis=2)   # [T,32]
    nsxgT = np.concatenate([-sxg.T, np.ones((1, T), np.float32)],
                           axis=0).astype(np.float16)                    # [33,T]
    # dnT_d[p, g*128+r] = dn[r, g*128+p]
    dnT = np.ascontiguousarray(
        svd_down.T.reshape(N_GROUPS, 128, RANK).transpose(1, 0, 2).reshape(128, IN)
    ).astype(np.float16)

    npad = PAD - SHARD
    in_maps = []
    for c in range(NCORES):
        sl = slice(c * SHARD, (c + 1) * SHARD)
        import ml_dtypes
        f8 = ml_dtypes.float8_e4m3
        w_c = np.concatenate([weight[sl].astype(f8),
                              np.zeros((npad, N_GROUPS, GROUP), f8)], axis=0)
        # w_d[i*128+p, g*128+c2] = w[i*128+c2, g, p]
        w_l = np.ascontiguousarray(
            w_c.reshape(NTILES, 128, N_GROUPS, 128).transpose(0, 3, 2, 1)
            .reshape(PAD, IN))
        sc_c = np.concatenate([scale[sl], np.zeros((npad, N_GROUPS), np.float32)], 0)
        # sc_l[p, i*32+g] = sc[i*128+p, g]; scB repeats each column T times
        sc_l = np.ascontiguousarray(
            sc_c.reshape(NTILES, 128, N_GROUPS).transpose(1, 0, 2)
            .reshape(128, NTILES * N_GROUPS))
        scB = np.repeat(sc_l.astype(np.float16), T, axis=1)   # [128, 22528]
        # zp/svd/bias terms ride in P[g=31], pre-divided by sc[o,31]
        sc0 = sc_c[:, N_GROUPS - 1].copy()
        sc0[sc0 == 0] = 1.0
        zp_c = np.concatenate([zero_point[sl],
                               np.zeros((npad, N_GROUPS), np.float32)], 0)
        bias_c = np.concatenate([bias[sl], np.zeros(npad, np.float32)])
        zpscT = np.ascontiguousarray(
            np.concatenate([(zp_c * sc_c) / sc0[:, None],
                            (bias_c / sc0)[:, None]], axis=1).T
        ).astype(np.float16)                                              # [33,PAD]
        up_c = np.concatenate([svd_up[sl], np.zeros((npad, RANK), np.float32)], 0)
        upT = np.ascontiguousarray((up_c / sc0[:, None]).T).astype(np.float16)
        in_maps.append(dict(
            w=w_l, xqp=xqp_l, scB=scB, zpscT=zpscT, nsxgT=nsxgT,
            upT=upT, dnT=dnT))
    return in_maps


def kernel(x, weight, scale, zero_point, svd_up, svd_down, bias):
    nc = _build()
    in_maps = _prep_inputs(x, weight, scale, zero_point, svd_up, svd_down, bias)
    _nc_cache["last_in_maps"] = in_maps
    from concourse.bass_utils import run_bass_kernel_spmd
    res = run_bass_kernel_spmd(nc, in_maps, core_ids=list(range(NCORES)))
    outs = [r["out"][:SHARD].astype(np.float32) for r in res.results]
    full = np.concatenate(outs, axis=0)                         # [OUT, T]
    return np.ascontiguousarray(full.T)[None].astype(np.float32)  # [1, T, OUT]
